# revision 39
# baseline (speedup 1.0000x reference)
"""Trainium2 Bass kernel for nn_Attention_65223373357517.

Computes, for s,q [B=16, L=1024, D=1024] (D = 2H, H=512):
    a  = einsum('bsd,btd->bst', s, q)
    b  = softmax(a, -1) @ q
    c  = softmax(a^T, -1) @ s
    s~ = heuristic(s, b);  q~ = heuristic(q, c)
with heuristic(x, y) = g*r + (1-g)*x,
    r = gelu_tanh([x, y, x*y, x-y] @ w_r.T + b_r)
    g = sigmoid ([x, y, x*y, x-y] @ w_g.T + b_g)

Strategy: data-parallel over batch (2 examples per core, 8 cores, no
collectives).  Host folds (x-y) into the x/y weight blocks (W1+W4, W2-W4,
W3), so the heuristic contraction is 3D = 3072.

Per batch on-chip, all phases keep the PE dense:
  A:  A = S Q^T fp16 matmuls (k-outer so MMs start after the first chunks
      land); row stats m1/d1 via fused exp+accum; l1 = m1 + ln d1.
      A^T computed by a second fp16 MM pass (Q S^T) straight from the
      resident S^T/Q^T tiles -- no PE-transpose barrier; stats m2/d2 and
      P1^T = exp(A^T - l1) consumed directly from PSUM.
  B:  b^T = Q_nat^T-contracted matmuls (fp16) with rhs P1^T; results written
      to fp8 "pair" tiles (DoubleRow layout) + fp16 scratch for the x*y
      products; then P2^T = exp(A - l2) and c^T likewise.
  C:  heuristic: fp8 DoubleRow matmuls (256-contraction per instruction,
      ~2x bf16 throughput).  Weights are prescaled x64 on the host to keep
      e4m3 out of subnormals; the activation applies scale=1/64.  ACT order
      groups gelu x4 then sigmoid x4 per m-strip to halve table swaps.
      Epilogue out = x + g*(r - x) on DVE/GPSIMD; next batch's S^T/Q^T
      loads are interleaved after each output strip.
"""

import numpy as np
import ml_dtypes

B, L, D = 16, 1024, 1024
NCORES = 8
BLOC = B // NCORES          # batches per core
NK = D // 128               # 128-chunks of the feature dim
NM = D // 128               # output-row chunks
KF = 3 * D // 128           # folded heuristic contraction chunks (24)
NPAIR = KF // 2             # DoubleRow pair chunks (12)
NJ = D // 256               # fp8 pair tiles per activation block (4)
NH = 2                      # 512-wide halves of a 1024 free dim
WS = 64.0                   # host weight prescale (fp8 subnormal avoidance)

_nc_cache = None


def _build():
    import concourse.tile as tile
    from concourse import bacc, mybir

    FP32 = mybir.dt.float32
    FP16 = mybir.dt.float16
    FP8 = mybir.dt.float8e4
    AF = mybir.ActivationFunctionType
    ALU = mybir.AluOpType
    AX = mybir.AxisListType
    DR = mybir.MatmulPerfMode.DoubleRow

    nc = bacc.Bacc("TRN2", target_bir_lowering=False, debug=False)

    sth_d = nc.dram_tensor("sth", [BLOC, D, L], FP16, kind="ExternalInput")
    qth_d = nc.dram_tensor("qth", [BLOC, D, L], FP16, kind="ExternalInput")
    snh_d = nc.dram_tensor("snh", [BLOC, L, D], FP16, kind="ExternalInput")
    qnh_d = nc.dram_tensor("qnh", [BLOC, L, D], FP16, kind="ExternalInput")
    sf8_d = nc.dram_tensor("sf8", [BLOC, NJ, 128, 2, L], FP8, kind="ExternalInput")
    qf8_d = nc.dram_tensor("qf8", [BLOC, NJ, 128, 2, L], FP8, kind="ExternalInput")
    wr_d = nc.dram_tensor("wr", [NM, 128, KF, 128], FP8, kind="ExternalInput")
    wg_d = nc.dram_tensor("wg", [NM, 128, KF, 128], FP8, kind="ExternalInput")
    brt_d = nc.dram_tensor("brt", [128, NM], FP32, kind="ExternalInput")
    bgt_d = nc.dram_tensor("bgt", [128, NM], FP32, kind="ExternalInput")
    outs_d = nc.dram_tensor("outs", [BLOC, D, L], FP32, kind="ExternalOutput")
    outq_d = nc.dram_tensor("outq", [BLOC, D, L], FP32, kind="ExternalOutput")
    # DRAM bounce buffers for the [128, NK] -> [1, L] stat transposes
    l1scr_d = nc.dram_tensor("l1scr", [NK, 128], FP32, kind="Internal")
    l2scr_d = nc.dram_tensor("l2scr", [NK, 128], FP32, kind="Internal")

    with tile.TileContext(nc) as tc:
        with (
            tc.tile_pool(name="prog", bufs=1) as Pp,
            tc.tile_pool(name="main", bufs=1) as P,
        ):
            brt = Pp.tile([128, NM], FP32, tag="brt", name="brt")
            nc.sync.dma_start(brt[:], brt_d[:])
            bgt = Pp.tile([128, NM], FP32, tag="bgt", name="bgt")
            nc.sync.dma_start(bgt[:], bgt_d[:])

            def load_stq_chunk(b, k):
                st = P.tile([128, L], FP16, tag=f"st{k}", name=f"st{b}_{k}")
                nc.sync.dma_start(st[:], sth_d[b, k * 128:(k + 1) * 128, :])
                qt = P.tile([128, L], FP16, tag=f"qt{k}", name=f"qt{b}_{k}")
                nc.sync.dma_start(qt[:], qth_d[b, k * 128:(k + 1) * 128, :])
                return st, qt

            stq_pre = [load_stq_chunk(0, k) for k in range(NK)]

            for b in range(BLOC):
                st = [t[0] for t in stq_pre]
                qt = [t[1] for t in stq_pre]
                stq_next = [None] * NK

                # phase-B lhsT (natural-layout s; c^T runs first) and fp8 x pairs
                sn = []
                for kt in range(NK):
                    t = P.tile([128, D], FP16, tag=f"nat{kt}", name=f"sn{b}_{kt}")
                    nc.sync.dma_start(t[:], snh_d[b, kt * 128:(kt + 1) * 128, :])
                    sn.append(t)
                xp_s = []
                xp_q = []
                for j in range(NJ):
                    t = P.tile([128, 2, L], FP8, tag=f"sf8{j}", name=f"sf8{b}_{j}")
                    nc.sync.dma_start(t[:], sf8_d[b, j])
                    xp_s.append(t)
                    t = P.tile([128, 2, L], FP8, tag=f"qf8{j}", name=f"qf8{b}_{j}")
                    nc.sync.dma_start(t[:], qf8_d[b, j])
                    xp_q.append(t)

                # f32: the b/c softmax numerators and denominators must see
                # identically-rounded logits or the peaked-softmax scale
                # error (~exp(logit rounding)) stops cancelling.
                AT = [P.tile([128, L], FP32, tag=f"AT{mt}", name=f"AT{b}_{mt}")
                      for mt in range(NK)]
                # per-half stats (a/b) later combined via log-sum-exp
                negm1 = [P.tile([128, NK], FP32, tag=f"negm1{i}",
                                name=f"negm1{b}_{i}") for i in range(NH)]
                d1 = [P.tile([128, NK], FP32, tag=f"d1{i}",
                             name=f"d1{b}_{i}") for i in range(NH)]
                negm2 = [P.tile([128, NK], FP32, tag=f"negm2{i}",
                                name=f"negm2{b}_{i}") for i in range(NH)]
                d2 = [P.tile([128, NK], FP32, tag=f"d2{i}",
                             name=f"d2{b}_{i}") for i in range(NH)]
                l1a = P.tile([128, NK], FP32, tag="l1a", name=f"l1a{b}")
                l2a = P.tile([128, NK], FP32, tag="l2a", name=f"l2a{b}")
                # l2row allocated first (phase T runs first); shared slot
                l2row = P.tile([1, L], FP32, tag="lrow", bufs=1,
                               name=f"l2row{b}")
                l1row = P.tile([1, L], FP32, tag="lrow", bufs=1,
                               name=f"l1row{b}")
                l1bc = P.tile([128, L], FP32, tag="l1bc", name=f"l1bc{b}")
                l2bc = P.tile([128, L], FP32, tag="l2bc", name=f"l2bc{b}")

                # ---- phase T (first): A^T = Q S^T, stats -> l2 ----
                # Computing A^T before A makes l2 available early, so P2^T
                # can be built per-tile as A drains and c^T starts with no
                # exposed stats chain; l1's chain then hides under c^T.
                def lse_combine(negm, d, la, pfx):
                    # l = m + ln(d_a e^{m_a-m} + d_b e^{m_b-m}), m = max
                    nmin = P.tile([128, NK], FP32, tag="cmb0", name=f"nm{pfx}")
                    nc.vector.tensor_tensor(
                        nmin[:], negm[0][:], negm[1][:], ALU.min)
                    dt = []
                    for i in range(NH):
                        df = P.tile([128, NK], FP32, tag=f"cmb{i}a",
                                    name=f"df{pfx}{i}")
                        nc.vector.tensor_sub(df[:], nmin[:], negm[i][:])
                        ef = P.tile([128, NK], FP32, tag=f"cmb{i}b",
                                    name=f"ef{pfx}{i}")
                        nc.scalar.activation(ef[:], df[:], AF.Exp)
                        dm = P.tile([128, NK], FP32, tag=f"cmb{i}c",
                                    name=f"dm{pfx}{i}")
                        nc.vector.tensor_mul(dm[:], d[i][:], ef[:])
                        dt.append(dm)
                    ds = P.tile([128, NK], FP32, tag="cmb4", name=f"ds{pfx}")
                    nc.vector.tensor_add(ds[:], dt[0][:], dt[1][:])
                    lnd = P.tile([128, NK], FP32, tag="cmb5", name=f"ln{pfx}")
                    nc.scalar.activation(lnd[:], ds[:], AF.Ln)
                    nc.vector.tensor_sub(la[:], lnd[:], nmin[:])

                with tc.tile_pool(name=f"psT{b}", bufs=1, space="PSUM") as PSt:
                    for mt in range(NK):
                        pat = PSt.tile([128, L], FP32, tag="pat", bufs=3,
                                       name=f"pat{b}_{mt}")
                        for h in range(NH):
                            sl = slice(h * 512, (h + 1) * 512)
                            for k in range(NK):
                                nc.tensor.matmul(
                                    pat[:, sl],
                                    qt[k][:, mt * 128:(mt + 1) * 128],
                                    st[k][:, h * 512:(h + 1) * 512],
                                    start=(k == 0), stop=(k == NK - 1))
                            # drain this half now; stats read the fp16 copy
                            nc.vector.tensor_copy(AT[mt][:, sl], pat[:, sl])
                            nc.vector.tensor_reduce(
                                negm2[h][:, mt:mt + 1], AT[mt][:, sl], AX.X,
                                ALU.max, negate=True)
                            e2 = P.tile([128, L], FP16, tag="scr16", bufs=2,
                                        name=f"e2{b}_{mt}{h}")
                            nc.scalar.activation(
                                e2[:, 0:512], AT[mt][:, sl], AF.Exp,
                                bias=negm2[h][:, mt:mt + 1],
                                accum_out=d2[h][:, mt:mt + 1])
                    lse_combine(negm2, d2, l2a, f"2_{b}")
                    # [128, NK] -> [1, L]: transpose via a DRAM bounce (the
                    # store iterates (p, ms) writing l2scr[ms, p]); no PE op.
                    nc.sync.dma_start(
                        l2scr_d[:, :].rearrange("m p -> p m"), l2a[:])
                    nc.sync.dma_start(
                        l2row[:1, :].rearrange("a (m p) -> a m p", p=128),
                        l2scr_d[:, :])
                    nc.gpsimd.partition_broadcast(l2bc[:], l2row[:])

                # ---- phase A: A = S Q^T; P2^T = exp(A - l2) as A drains ----
                p2t = [P.tile([128, L], FP16, tag=f"pt2_{ms}",
                              name=f"p2t{b}_{ms}") for ms in range(NK)]
                with tc.tile_pool(name=f"psA{b}", bufs=1, space="PSUM") as PSa:
                    # ms-outer / k-inner: a bank completes every ~1.7us so
                    # its drain (stats + P2^T half) pipelines under the next
                    # bank's matmuls instead of bursting at the half's end.
                    for h in range(NH):
                        sl = slice(h * 512, (h + 1) * 512)
                        for ms in range(NK):
                            pa = PSa.tile([128, 512], FP32, tag="pa", bufs=4,
                                          name=f"pa{b}_{h}_{ms}")
                            for k in range(NK):
                                nc.tensor.matmul(
                                    pa[:],
                                    st[k][:, ms * 128:(ms + 1) * 128],
                                    qt[k][:, sl],
                                    start=(k == 0), stop=(k == NK - 1))
                            nc.vector.tensor_reduce(
                                negm1[h][:, ms:ms + 1], pa[:], AX.X,
                                ALU.max, negate=True)
                            esc = P.tile([128, L], FP16, tag="scr16",
                                         bufs=2, name=f"esc{b}_{ms}{h}")
                            nc.scalar.activation(
                                esc[:, 0:512], pa[:], AF.Exp,
                                bias=negm1[h][:, ms:ms + 1],
                                accum_out=d1[h][:, ms:ms + 1])
                            sh2 = P.tile([128, 512], FP32, tag="sh",
                                         bufs=2, name=f"sh2{b}_{ms}{h}")
                            nc.vector.tensor_sub(sh2[:], pa[:], l2bc[:, sl])
                            nc.scalar.activation(
                                p2t[ms][:, sl], sh2[:], AF.Exp)
                    lse_combine(negm1, d1, l1a, f"1_{b}")
                    nc.sync.dma_start(
                        l1scr_d[:, :].rearrange("m p -> p m"), l1a[:])
                    nc.sync.dma_start(
                        l1row[:1, :].rearrange("a (m p) -> a m p", p=128),
                        l1scr_d[:, :])
                    nc.gpsimd.partition_broadcast(l1bc[:], l1row[:])

                # ---- phase B: b^T / c^T, fp8 pair tiles + x*y products ----
                yp_s = [P.tile([128, 2, L], FP8, tag=f"yps{j}",
                               name=f"yps{b}_{j}") for j in range(NJ)]
                yp_q = [P.tile([128, 2, L], FP8, tag=f"ypq{j}",
                               name=f"ypq{b}_{j}") for j in range(NJ)]
                zp_s = [P.tile([128, 2, L], FP8, tag=f"zps{j}",
                               name=f"zps{b}_{j}") for j in range(NJ)]
                zp_q = [P.tile([128, 2, L], FP8, tag=f"zpq{j}",
                               name=f"zpq{b}_{j}") for j in range(NJ)]

                with tc.tile_pool(name=f"psB{b}", bufs=1, space="PSUM") as PSb:
                    qn = []

                    def load_qn(kt):
                        t = P.tile([128, D], FP16, tag=f"nat{kt}",
                                   name=f"qn{b}_{kt}")
                        nc.sync.dma_start(
                            t[:], qnh_d[b, kt * 128:(kt + 1) * 128, :])
                        qn.append(t)

                    def stage2(lhs, pt, xt, yp, zp, nm):
                        for mdg in range(2):
                            mds = range(4 * mdg, 4 * mdg + 4)
                            pbs = [PSb.tile([128, L], FP32, tag="pb", bufs=4,
                                            name=f"pb{b}{nm}_{md}")
                                   for md in mds]
                            for kt in range(NK):
                                for i, md in enumerate(mds):
                                    for h in range(NH):
                                        nc.tensor.matmul(
                                            pbs[i][:, h * 512:(h + 1) * 512],
                                            lhs[kt][:, md * 128:(md + 1) * 128],
                                            pt[kt][:, h * 512:(h + 1) * 512],
                                            start=(kt == 0), stop=(kt == NK - 1))
                                if nm == "q" and mdg == 1:
                                    # refill the nat slot right after c^T's
                                    # last use of sn[kt]
                                    load_qn(kt)
                            for i, md in enumerate(mds):
                                j, jj = md // 2, md % 2
                                nc.scalar.copy(yp[j][:, jj, :], pbs[i][:])
                                nc.vector.tensor_mul(
                                    zp[j][:, jj, :], xt[md][:], pbs[i][:])

                    # P1^T issued first so its exps are not queued behind
                    # c^T's PSUM-drain copies on the scalar engine.
                    p1t = []
                    for mt in range(NK):
                        pt_ = P.tile([128, L], FP16, tag=f"pt1_{mt}",
                                     name=f"p1t{b}_{mt}")
                        for hh in range(NH):
                            sl = slice(hh * 512, (hh + 1) * 512)
                            sh = P.tile([128, 512], FP32, tag="sh", bufs=2,
                                        name=f"sh{b}_{mt}{hh}")
                            nc.vector.tensor_sub(
                                sh[:], AT[mt][:, sl], l1bc[:, sl])
                            nc.scalar.activation(pt_[:, sl], sh[:], AF.Exp)
                        p1t.append(pt_)

                    # prefetch the first heuristic weight strips so phase C's
                    # first matmuls are not gated on their DMA
                    wts = {}
                    for m in range(2):
                        wrt = P.tile([128, KF, 128], FP8, tag="wr8", bufs=2,
                                     name=f"wrt{b}_{m}")
                        nc.sync.dma_start(wrt[:], wr_d[m])
                        wgt = P.tile([128, KF, 128], FP8, tag="wg8", bufs=2,
                                     name=f"wgt{b}_{m}")
                        nc.sync.dma_start(wgt[:], wg_d[m])
                        wts[m] = (wrt, wgt)

                    # c^T first: its P2^T inputs are ready at A's end; l1's
                    # chain and P1^T hide under these matmuls.
                    stage2(sn, p2t, qt, yp_q, zp_q, "q")
                    stage2(qn, p1t, st, yp_s, zp_s, "s")

                # ---- phase C: heuristic, fp8 DoubleRow ----
                with tc.tile_pool(name=f"psC{b}", bufs=1, space="PSUM") as PSc:
                    for m in range(NM):
                        if m in wts:
                            wrt, wgt = wts[m]
                        else:
                            wrt = P.tile([128, KF, 128], FP8, tag="wr8",
                                         bufs=2, name=f"wrt{b}_{m}")
                            nc.sync.dma_start(wrt[:], wr_d[m])
                            wgt = P.tile([128, KF, 128], FP8, tag="wg8",
                                         bufs=2, name=f"wgt{b}_{m}")
                            nc.sync.dma_start(wgt[:], wg_d[m])

                        res = {}
                        for tag, xp, yp, zp in (("q", xp_q, yp_q, zp_q),
                                                ("s", xp_s, yp_s, zp_s)):
                            pairs = xp + yp + zp
                            for br, w in (("r", wrt), ("g", wgt)):
                                ps = [PSc.tile([128, 512], FP32, tag="rg",
                                               bufs=8,
                                               name=f"p{br}{b}_{m}{tag}{h}")
                                      for h in range(NH)]
                                for j in range(NPAIR):
                                    for h in range(NH):
                                        nc.tensor.matmul(
                                            ps[h][:],
                                            w[:, 2 * j:2 * j + 2, :],
                                            pairs[j][:, :, h * 512:(h + 1) * 512],
                                            start=(j == 0),
                                            stop=(j == NPAIR - 1),
                                            perf_mode=DR)
                                res[(tag, br)] = ps

                        acts = {}
                        for br, fn, bias in (("r", AF.Gelu_apprx_tanh, brt),
                                             ("g", AF.Sigmoid, bgt)):
                            for tag in ("q", "s"):
                                o = P.tile([128, L], FP16, tag=f"{br}sb",
                                           bufs=2, name=f"{br}sb{b}_{m}{tag}")
                                for h in range(NH):
                                    nc.scalar.activation(
                                        o[:, h * 512:(h + 1) * 512],
                                        res[(tag, br)][h][:], fn,
                                        bias=bias[:, m:m + 1], scale=1.0 / WS)
                                acts[(tag, br)] = o

                        for tag, xt, outd in (("q", qt, outq_d),
                                              ("s", st, outs_d)):
                            t1 = P.tile([128, L], FP16, tag="scr16", bufs=2,
                                        name=f"t1{b}_{m}{tag}")
                            nc.vector.tensor_sub(
                                t1[:], acts[(tag, "r")][:], xt[m][:])
                            t2 = P.tile([128, L], FP16, tag="scr16", bufs=2,
                                        name=f"t2{b}_{m}{tag}")
                            # last strip: keep the tail off the slow gpsimd
                            mul_eng = (nc.vector if m == NM - 1 else nc.gpsimd)
                            mul_eng.tensor_mul(
                                t2[:], acts[(tag, "g")][:], t1[:])
                            osb = P.tile([128, L], FP32, tag="ep3", bufs=1,
                                         name=f"osb{b}_{m}{tag}")
                            nc.vector.tensor_add(osb[:], t2[:], xt[m][:])
                            nc.sync.dma_start(
                                outd[b, m * 128:(m + 1) * 128, :], osb[:])

                        # prefetch next batch's S^T/Q^T chunk m right after its
                        # last consumer (this m's epilogue) in program order
                        if b + 1 < BLOC:
                            stq_next[m] = load_stq_chunk(b + 1, m)

                stq_pre = stq_next

    nc.compile()
    return nc


def _get_nc():
    global _nc_cache
    if _nc_cache is None:
        _nc_cache = _build()
    return _nc_cache


def _prep_inputs(s, q, w_r, b_r, w_g, b_g):
    f16 = np.float16
    f8 = ml_dtypes.float8_e4m3
    s = np.asarray(s, dtype=np.float32)
    q = np.asarray(q, dtype=np.float32)
    w_r = np.asarray(w_r, dtype=np.float32)
    w_g = np.asarray(w_g, dtype=np.float32)
    b_r = np.asarray(b_r, dtype=np.float32)
    b_g = np.asarray(b_g, dtype=np.float32)

    st = np.ascontiguousarray(s.transpose(0, 2, 1))
    qt = np.ascontiguousarray(q.transpose(0, 2, 1))
    sth = st.astype(f16)
    qth = qt.astype(f16)
    snh = s.astype(f16)
    qnh = q.astype(f16)
    sf8 = np.ascontiguousarray(
        st.reshape(B, NJ, 2, 128, L).transpose(0, 1, 3, 2, 4)).astype(f8)
    qf8 = np.ascontiguousarray(
        qt.reshape(B, NJ, 2, 128, L).transpose(0, 1, 3, 2, 4)).astype(f8)

    def pack_w(w):
        W1, W2, W3, W4 = (w[:, i * D:(i + 1) * D] for i in range(4))
        eff = np.concatenate([W1 + W4, W2 - W4, W3], axis=1)  # [D, 3D]
        wt = eff.T  # [3D, D]
        pk = wt.reshape(KF, 128, NM, 128).transpose(2, 1, 0, 3)  # [m, f, k, o]
        return np.ascontiguousarray(pk * WS).astype(f8)

    wr_pack = pack_w(w_r)
    wg_pack = pack_w(w_g)
    brt = np.ascontiguousarray(b_r.reshape(NM, 128).T)
    bgt = np.ascontiguousarray(b_g.reshape(NM, 128).T)

    in_maps = []
    for c in range(NCORES):
        sl = slice(BLOC * c, BLOC * (c + 1))
        in_maps.append({
            "sth": sth[sl], "qth": qth[sl],
            "snh": snh[sl], "qnh": qnh[sl],
            "sf8": sf8[sl], "qf8": qf8[sl],
            "wr": wr_pack, "wg": wg_pack,
            "brt": brt, "bgt": bgt,
        })
    return in_maps


def run(inputs, trace=False, tmpdir=None):
    """Execute on 8 NeuronCores; returns ((s_tilde, q_tilde), BassKernelResults)."""
    from concourse.bass_utils import run_bass_kernel_spmd

    in_maps = _prep_inputs(
        inputs["s"], inputs["q"], inputs["w_r"], inputs["b_r"],
        inputs["w_g"], inputs["b_g"])
    nc = _get_nc()
    res = run_bass_kernel_spmd(nc, in_maps, list(range(NCORES)), trace=trace,
                               tmpdir=tmpdir)
    s_t = np.empty((B, L, D), np.float32)
    q_t = np.empty((B, L, D), np.float32)
    for c in range(NCORES):
        sl = slice(BLOC * c, BLOC * (c + 1))
        s_t[sl] = res.results[c]["outs"].transpose(0, 2, 1)
        q_t[sl] = res.results[c]["outq"].transpose(0, 2, 1)
    return (s_t, q_t), res


def kernel(s, q, w_r, b_r, w_g, b_g, s_mask=None, q_mask=None):
    # s_mask / q_mask are all-ones in this problem; the additive mask term
    # (1 - m1*m2) * NEG_INF is identically zero, so they are unused.
    out, _ = run({"s": s, "q": q, "w_r": w_r, "b_r": b_r,
                  "w_g": w_g, "b_g": b_g})
    return out
